# revision 17
# baseline (speedup 1.0000x reference)
"""Trainium2 Bass kernel for CausalSelfAttentionLayer (ragged uniform, B=4 L=1024 C=512).

Sharding over 8 NeuronCores: core c -> sequence b=c//2, head-group g=c%2 (8 of 16
heads).  Per core: LN (bn_stats on DVE, apply on ACT via Identity(scale=rstd,
bias=-mu*rstd)), PE transposes to feature-major xT, f32r QKV matmuls with a K=4
rank-1 bias correction, per-head attention with transposed probabilities
(scoresT = kT.T @ qT), constant-shift exp on ACT, host-built causal mask, AV
with token-major V, row-sums via ones-matmul, bf16 row-parallel c_proj
partials, ReduceScatter over core pairs.  Host concatenates the 8 [512, 2048]
shards.  Pair-0 QKV emission is interleaved into phase A so the PE's in-order
stream has work while LN/transposes stream.
"""
import sys

for _p in ("/opt/trn_rl_repo", "/root/.axon_site/_ro/trn_rl_repo"):
    if _p not in sys.path:
        sys.path.append(_p)

from contextlib import ExitStack

import numpy as np

import concourse.bass as bass
import concourse.mybir as mybir
from concourse.bass import _add_dep_helper
import concourse.tile as tile
from concourse import bacc
from concourse.bass_utils import run_bass_kernel_spmd

B, L, C, H, D = 4, 1024, 512, 2048, 128
NHL = 8          # heads per core
NPAIR = 4        # head pairs per core
T = L            # tokens per core (one sequence)
EPS = 1e-5
f32 = mybir.dt.float32
f32r = mybir.dt.float32r
bf16 = mybir.dt.bfloat16

_CACHE = {}


def _build(debug=False, collective=True, nbody=1):
    nc = bacc.Bacc("TRN2", target_bir_lowering=False, debug=False, num_devices=8)

    x_t = nc.dram_tensor("x", [T, H], f32r, kind="ExternalInput")
    wq_t = nc.dram_tensor("wq", [H, NHL * D], f32r, kind="ExternalInput")
    wk_t = nc.dram_tensor("wk", [H, NHL * D], f32r, kind="ExternalInput")
    wv_t = nc.dram_tensor("wv", [H, NHL * D], f32r, kind="ExternalInput")
    rowsa_t = nc.dram_tensor("rowsa", [4, 3 * NHL * D], f32r, kind="ExternalInput")
    rowsb_t = nc.dram_tensor("rowsb", [4, T], f32r, kind="ExternalInput")
    kc_t = nc.dram_tensor("kc", [C, NHL, D], f32r, kind="ExternalInput")
    vc_t = nc.dram_tensor("vc", [C, NHL, D], f32r, kind="ExternalInput")
    pw_t = nc.dram_tensor("pw", [NHL * D, H], bf16, kind="ExternalInput")
    pb_t = nc.dram_tensor("pb", [1, H], f32, kind="ExternalInput")
    id_t = nc.dram_tensor("ident", [128, 128], f32r, kind="ExternalInput")
    mask_t = nc.dram_tensor("mask", [128, 896], f32r, kind="ExternalInput")
    onc_t = nc.dram_tensor("onescol", [128, 1], f32r, kind="ExternalInput")
    out_t = nc.dram_tensor("out", [T // 2, H], f32, kind="ExternalOutput")
    if debug:
        dbg_xt = nc.dram_tensor("dbg_xt", [128, 16, T], f32, kind="ExternalOutput")
        dbg_q = nc.dram_tensor("dbg_q", [128, 2, T], f32, kind="ExternalOutput")
        dbg_k = nc.dram_tensor("dbg_k", [128, 2, T], f32, kind="ExternalOutput")
        dbg_v = nc.dram_tensor("dbg_v", [128, 8, 256], f32, kind="ExternalOutput")
        dbg_ot = nc.dram_tensor("dbg_ot", [128, NHL, T], bf16,
                                kind="ExternalOutput")
        dbg_pp = nc.dram_tensor("dbg_pp", [T, H], f32, kind="ExternalOutput")

    Exp = mybir.ActivationFunctionType.Exp
    Sqrt = mybir.ActivationFunctionType.Sqrt
    Ident = mybir.ActivationFunctionType.Identity
    mult = mybir.AluOpType.mult

    with tile.TileContext(nc) as tc, ExitStack() as _stk:
        def _pool(name, bufs, **kw):
            return _stk.enter_context(tc.tile_pool(name=name, bufs=bufs, **kw))
        cst = _pool("cst", 1)
        xload = _pool("xload", 4)
        st = _pool("st", 8)
        big = _pool("big", 1)
        wp = _pool("wp", 4)
        qkvp = _pool("qkv", 3)
        kvc = _pool("kvc", 2)
        kcnp = _pool("kcn", 2)
        ptp = _pool("ptp", 2)
        rb = _pool("rb", 1)
        otp = _pool("otp", 1)
        po = _pool("po", 2)
        ps = _pool("ps", 8, space="PSUM")
        dram = _pool("dram", 1, space="DRAM")

        # ---- constants ----
        ident = cst.tile([128, 128], f32r)
        nc.sync.dma_start(out=ident, in_=id_t.ap())
        mask = cst.tile([128, 896], f32r)
        nc.sync.dma_start(out=mask, in_=mask_t.ap())
        rowsa = cst.tile([4, 3 * NHL * D], f32r)   # 0, const, 0, 0
        nc.sync.dma_start(out=rowsa, in_=rowsa_t.ap())
        rowsb = cst.tile([4, T], f32r)             # 0, ones, 0, 0
        nc.sync.dma_start(out=rowsb, in_=rowsb_t.ap())
        onescol = cst.tile([128, 1], f32r)
        nc.sync.dma_start(out=onescol, in_=onc_t.ap())
        epst = cst.tile([128, 1], f32)
        nc.vector.memset(epst, EPS)

        partial = dram.tile([T, H], f32)
        rsout = dram.tile([T // 2, H], f32)

        for _rep in range(nbody):
            xT = big.tile([128, 16, T], f32r, tag="big")

            def ln_chunk(tch):
                """LN one 128-token chunk and transpose into xT."""
                xh = [xload.tile([128, 1024], f32r, tag="x", name=f"xh{i_}")
                      for i_ in range(2)]
                stats = st.tile([128, 4, 6], f32, tag="stats")
                for half in range(2):
                    d = nc.sync.dma_start(
                        out=xh[half],
                        in_=x_t.ap()[tch * 128:(tch + 1) * 128,
                                     half * 1024:(half + 1) * 1024])
                    x_dmas.append(d)
                    for sg in range(2):
                        nc.vector.bn_stats(
                            out=stats[:, half * 2 + sg, :],
                            in_=xh[half][:, sg * 512:(sg + 1) * 512].bitcast(f32))
                mv = st.tile([128, 2], f32, tag="mv")
                nc.vector.bn_aggr(out=mv, in_=stats)
                stdt = st.tile([128, 1], f32, tag="std")
                nc.scalar.activation(stdt, mv[:, 1:2], Sqrt, bias=epst[:, 0:1])
                rstd = st.tile([128, 1], f32, tag="rstd")
                nc.vector.reciprocal(rstd, stdt)
                m2p = st.tile([128, 1], f32, tag="m2p")
                nc.vector.tensor_scalar(
                    out=m2p, in0=mv[:, 0:1],
                    scalar1=rstd[:, 0:1], scalar2=-1.0, op0=mult, op1=mult)
                for half in range(2):
                    nc.scalar.activation(
                        xh[half][:], xh[half][:].bitcast(f32), Ident,
                        bias=m2p[:, 0:1], scale=rstd[:, 0:1])
                for hch in range(16):
                    tp = ps.tile([128, 512], f32, tag="ps", name="tp")
                    nc.tensor.transpose(
                        tp[:, 0:128].bitcast(f32r),
                        xh[hch // 8][:, (hch % 8) * 128:(hch % 8 + 1) * 128],
                        ident)
                    nc.vector.tensor_copy(
                        out=xT[:, hch, tch * 128:(tch + 1) * 128],
                        in_=tp[:, 0:128].bitcast(f32r))

            def load_w(wdram, hp, after=()):
                wh = []
                for half in range(2):
                    w = wp.tile([128, 8, 256], f32r, tag="w", name=f"w{half}")
                    wh.append(w)
                    d = nc.scalar.dma_start(
                        out=w,
                        in_=wdram.ap()[half * 1024:(half + 1) * 1024,
                                       hp * 256:(hp + 1) * 256].rearrange(
                            "(c p) n -> p c n", p=128))
                    for a in after:
                        _add_dep_helper(d.ins, a.ins,
                                        reason="delay W behind x loads")
                return wh

            def emit_qk(dst, wh, colbase, jh, tt):
                pq = ps.tile([128, 512], f32, tag="ps", name="pq")
                for half in range(2):
                    for hc in range(8):
                        nc.tensor.matmul(
                            pq[:],
                            wh[half][:, hc, jh * 128:(jh + 1) * 128],
                            xT[:, half * 8 + hc, tt * 512:(tt + 1) * 512],
                            start=(half == 0 and hc == 0), stop=False)
                nc.tensor.matmul(
                    pq[:],
                    rowsa[0:4, colbase + jh * 128:colbase + (jh + 1) * 128],
                    rowsb[0:4, tt * 512:(tt + 1) * 512],
                    start=False, stop=True)
                nc.vector.tensor_copy(
                    out=dst[:, jh, tt * 512:(tt + 1) * 512],
                    in_=pq[:].bitcast(f32r))

            def emit_v(vtok, whv, vcol, tch):
                pv = ps.tile([128, 512], f32, tag="ps", name="pv")
                for half in range(2):
                    for hc in range(8):
                        nc.tensor.matmul(
                            pv[:, 0:256],
                            xT[:, half * 8 + hc, tch * 128:(tch + 1) * 128],
                            whv[half][:, hc, :],
                            start=(half == 0 and hc == 0), stop=False)
                nc.tensor.matmul(
                    pv[:, 0:256],
                    rowsb[0:4, tch * 128:(tch + 1) * 128],
                    rowsa[0:4, vcol:vcol + 256],
                    start=False, stop=True)
                nc.vector.tensor_copy(out=vtok[:, tch, :],
                                      in_=pv[:, 0:256].bitcast(f32r))

            def load_cache(hp):
                kcT = kvc.tile([128, 2, C], f32r, tag="kv", name="kcT")
                for pc in range(4):
                    kcn = kcnp.tile([128, 2, 128], f32r, tag="kcn", name="kcn")
                    nc.scalar.dma_start(
                        out=kcn,
                        in_=kc_t.ap()[pc * 128:(pc + 1) * 128,
                                      hp * 2:hp * 2 + 2, :])
                    tp = ps.tile([128, 512], f32, tag="ps", name="tpc")
                    for hl in range(2):
                        nc.tensor.transpose(
                            tp[:, hl * 128:(hl + 1) * 128].bitcast(f32r),
                            kcn[:, hl, :], ident)
                    for hl in range(2):
                        nc.vector.tensor_copy(
                            out=kcT[:, hl, pc * 128:(pc + 1) * 128],
                            in_=tp[:, hl * 128:(hl + 1) * 128].bitcast(f32r))
                vcp = kvc.tile([128, 4, 2, 128], f32r, tag="kv", name="vcp")
                nc.scalar.dma_start(
                    out=vcp,
                    in_=vc_t.ap()[:, hp * 2:hp * 2 + 2, :].rearrange(
                        "(c p) h d -> p c h d", p=128))
                return kcT, vcp

            def emit_attn(outT, qT, kT, vtok, kcT, vcp, hp, hl, qt):
                h = hp * 2 + hl
                po_ps = ps.tile([128, 512], f32, tag="ps", name="po_ps")
                sm_ps = ps.tile([128, 512], f32, tag="ps", name="sm_ps")
                chunks = [("c", pc, None) for pc in range(4)]
                for kf in range(8):
                    dlt = qt * 512 - kf * 128
                    if dlt <= -512:
                        continue
                    chunks.append(("f", kf, dlt + 384 if dlt < 127 else None))
                first = True
                for kind, idx, ms in chunks:
                    sc_ps = ps.tile([128, 512], f32, tag="ps", name="sc_ps")
                    lhs = (kcT[:, hl, idx * 128:(idx + 1) * 128] if kind == "c"
                           else kT[:, hl, idx * 128:(idx + 1) * 128])
                    nc.tensor.matmul(
                        sc_ps[:], lhs, qT[:, hl, qt * 512:(qt + 1) * 512],
                        start=True, stop=True)
                    pt = ptp.tile([128, 512], f32r, tag="pt", name="pt")
                    nc.scalar.activation(pt[:], sc_ps[:], Exp)
                    if ms is not None:
                        nc.vector.tensor_mul(pt[:], pt[:], mask[:, ms:ms + 512])
                    vchunk = (vcp[:, idx, hl, :] if kind == "c"
                              else vtok[:, idx, hl * 128:(hl + 1) * 128])
                    nc.tensor.matmul(po_ps[:], vchunk, pt[:],
                                     start=first, stop=False)
                    nc.tensor.matmul(sm_ps[0:1, :], onescol, pt[:],
                                     start=first, stop=False)
                    first = False
                rc = st.tile([1, 512], f32, tag="recip", bufs=2)
                nc.vector.reciprocal(rc, sm_ps[0:1, :])
                rcb = rb.tile([128, 512], f32, tag="rcb", name="rcb")
                nc.gpsimd.partition_broadcast(rcb[:], rc[:])
                nc.vector.tensor_mul(
                    out=outT[:, h, qt * 512:(qt + 1) * 512],
                    in0=po_ps[:], in1=rcb[:])

            # ---- phase A interleaved with pair-0 QKV ----
            x_dmas = []
            for tch in range(2):
                ln_chunk(tch)
            first_x = list(x_dmas)
            wh_q = load_w(wq_t, 0, after=first_x)
            wh_v = load_w(wv_t, 0, after=first_x)
            for tch in range(2, 4):
                ln_chunk(tch)
            outT = otp.tile([128, NHL, T], bf16, tag="outT")
            qT0 = qkvp.tile([128, 2, T], f32r, tag="qkv", name="qT0")
            vtok0 = qkvp.tile([128, 8, 256], f32r, tag="qkv", name="vtok0")
            for jh in range(2):
                emit_qk(qT0, wh_q, 0, jh, 0)
            for tch in range(4):
                emit_v(vtok0, wh_v, 2048, tch)
            for tch in range(4, 8):
                ln_chunk(tch)
            for jh in range(2):
                emit_qk(qT0, wh_q, 0, jh, 1)
            for tch in range(4, 8):
                emit_v(vtok0, wh_v, 2048, tch)
            wh_k = load_w(wk_t, 0)
            kT0 = qkvp.tile([128, 2, T], f32r, tag="qkv", name="kT0")
            for jh in range(2):
                for tt in range(2):
                    emit_qk(kT0, wh_k, 1024, jh, tt)

            if debug:
                nc.sync.dma_start(out=dbg_xt.ap(), in_=xT[:].bitcast(f32))
                nc.sync.dma_start(out=dbg_q.ap(), in_=qT0[:].bitcast(f32))
                nc.sync.dma_start(out=dbg_k.ap(), in_=kT0[:].bitcast(f32))
                nc.sync.dma_start(out=dbg_v.ap(), in_=vtok0[:].bitcast(f32))

            # ---- phase B ----
            for hp in range(NPAIR):
                if hp == 0:
                    qT, kT, vtok = qT0, kT0, vtok0
                else:
                    wh = load_w(wq_t, hp)
                    qT = qkvp.tile([128, 2, T], f32r, tag="qkv", name="qT")
                    for jh in range(2):
                        for tt in range(2):
                            emit_qk(qT, wh, hp * 256, jh, tt)
                    wh = load_w(wk_t, hp)
                    kT = qkvp.tile([128, 2, T], f32r, tag="qkv", name="kT")
                    for jh in range(2):
                        for tt in range(2):
                            emit_qk(kT, wh, 1024 + hp * 256, jh, tt)
                    wh = load_w(wv_t, hp)
                    vtok = qkvp.tile([128, 8, 256], f32r, tag="qkv", name="vtok")
                    for tch in range(8):
                        emit_v(vtok, wh, 2048 + hp * 256, tch)

                if hp == NPAIR - 1:
                    pw_sb = big.tile([128, NHL, H], bf16, tag="big")
                    for hq_ in range(4):
                        nc.scalar.dma_start(
                            out=pw_sb[:, hq_ * 2:(hq_ + 1) * 2, :],
                            in_=pw_t.ap()[hq_ * 256:(hq_ + 1) * 256, :
                                          ].rearrange("(h p) n -> p h n", p=128))
                    pbb = wp.tile([128, H], f32, tag="w", name="pbb")
                    nc.gpsimd.dma_start(
                        out=pbb,
                        in_=bass.AP(tensor=pb_t.ap().tensor, offset=0,
                                    ap=[[0, 128], [1, H]]))

                kcT, vcp = load_cache(hp)
                for hl in range(2):
                    for qt in range(2):
                        emit_attn(outT, qT, kT, vtok, kcT, vcp, hp, hl, qt)

            if debug:
                nc.sync.dma_start(out=dbg_ot.ap(), in_=outT[:])

            # ---- phase C: c_proj partials (bf16) ----
            for tch in range(8):
                for ht in range(4):
                    pp = ps.tile([128, 512], f32, tag="ps", name="pp")
                    for h in range(NHL):
                        nc.tensor.matmul(
                            pp[:],
                            outT[:, h, tch * 128:(tch + 1) * 128],
                            pw_sb[:, h, ht * 512:(ht + 1) * 512],
                            start=(h == 0), stop=(h == NHL - 1))
                    ev = po.tile([128, 512], f32, tag="po", name="ev")
                    nc.vector.tensor_add(ev[:], pp[:],
                                         pbb[:, ht * 512:(ht + 1) * 512])
                    nc.sync.dma_start(
                        out=partial[tch * 128:(tch + 1) * 128,
                                    ht * 512:(ht + 1) * 512],
                        in_=ev[:])

            if debug:
                nc.sync.dma_start(out=dbg_pp.ap(), in_=partial[:, :])

        # ---- phase D: ReduceScatter over pairs, write out ----
        if collective:
            nc.gpsimd.collective_compute(
                "ReduceScatter",
                mybir.AluOpType.add,
                replica_groups=[[0, 1], [2, 3], [4, 5], [6, 7]],
                ins=[partial.opt()],
                outs=[rsout.opt()],
            )
            nc.sync.dma_start(out=out_t.ap(), in_=rsout[:, :])
        else:
            nc.sync.dma_start(out=out_t.ap(), in_=partial[0:T // 2, :])

    nc.compile()
    return nc


def _host_prep(inputs):
    import ml_dtypes
    hidden = np.ascontiguousarray(np.asarray(inputs["hidden_states"],
                                             dtype=np.float32))
    k_cache = np.asarray(inputs["k_cache"], dtype=np.float32)
    v_cache = np.asarray(inputs["v_cache"], dtype=np.float32)
    ln_w = np.asarray(inputs["ln_w"], dtype=np.float32)
    ln_b = np.asarray(inputs["ln_b"], dtype=np.float32)
    attn_w = np.asarray(inputs["attn_w"], dtype=np.float32)
    attn_b = np.asarray(inputs["attn_b"], dtype=np.float32)
    proj_w = np.asarray(inputs["proj_w"], dtype=np.float32)
    proj_b = np.asarray(inputs["proj_b"], dtype=np.float32)

    scale = np.float32(1.0 / np.sqrt(D))
    wln = attn_w * ln_w[:, None]                  # [H, 6144]
    cfull = ln_b @ attn_w + attn_b                # [6144]
    ident = np.eye(128, dtype=np.float32)
    cc = np.arange(896)[None, :] - 384
    mask = (np.arange(128)[:, None] <= cc).astype(np.float32)
    onescol = np.ones((128, 1), dtype=np.float32)
    rowsb = np.zeros((4, T), dtype=np.float32)
    rowsb[1] = 1.0

    in_maps = []
    for c in range(8):
        b, g = c // 2, c % 2
        hsl = slice(g * NHL, (g + 1) * NHL)
        qsl = slice(g * 1024, (g + 1) * 1024)
        ksl = slice(2048 + g * 1024, 2048 + (g + 1) * 1024)
        vsl = slice(4096 + g * 1024, 4096 + (g + 1) * 1024)
        wq = np.ascontiguousarray(wln[:, qsl]) * scale
        wk = np.ascontiguousarray(wln[:, ksl])
        wv = np.ascontiguousarray(wln[:, vsl])
        const = np.concatenate([cfull[qsl] * scale, cfull[ksl], cfull[vsl]])
        rowsa = np.zeros((4, 3072), dtype=np.float32)
        rowsa[1] = const
        in_maps.append({
            "x": np.ascontiguousarray(hidden[b * L:(b + 1) * L]),
            "wq": wq, "wk": wk, "wv": wv,
            "rowsa": rowsa, "rowsb": rowsb,
            "kc": np.ascontiguousarray(k_cache[b, :C, hsl, :]),
            "vc": np.ascontiguousarray(v_cache[b, :C, hsl, :]),
            "pw": np.ascontiguousarray(proj_w[qsl, :]).astype(ml_dtypes.bfloat16),
            "pb": (proj_b if g == 0 else np.zeros_like(proj_b)
                   ).reshape(1, H).astype(np.float32),
            "ident": ident, "mask": mask, "onescol": onescol,
        })
    return in_maps


def kernel(**inputs) -> np.ndarray:
    if "nc" not in _CACHE:
        _CACHE["nc"] = _build()
    nc = _CACHE["nc"]
    in_maps = _host_prep(inputs)
    res = run_bass_kernel_spmd(nc, in_maps, list(range(8)))
    out = np.concatenate([res.results[c]["out"] for c in range(8)], axis=0)
    return out.astype(np.float32)


# revision 19
# speedup vs baseline: 27.0115x; 27.0115x over previous
"""Trainium2 Bass kernel for CausalSelfAttentionLayer (ragged uniform, B=4 L=1024 C=512).

Sharding over 8 NeuronCores: core c -> sequence b=c//2, head-group g=c%2 (8 of 16
heads).  Per core: LN (bn_stats on DVE, apply on ACT via Identity(scale=rstd,
bias=-mu*rstd)), PE transposes to feature-major xT, f32r QKV matmuls with a K=4
rank-1 bias correction, per-head attention with transposed probabilities
(scoresT = kT.T @ qT), constant-shift exp on ACT, host-built causal mask, AV
with token-major V, row-sums via ones-matmul, bf16 row-parallel c_proj
partials, ReduceScatter over core pairs.  Host concatenates the 8 [512, 2048]
shards.  Pair-0 QKV emission is interleaved into phase A so the PE's in-order
stream has work while LN/transposes stream.
"""
import sys

for _p in ("/opt/trn_rl_repo", "/root/.axon_site/_ro/trn_rl_repo"):
    if _p not in sys.path:
        sys.path.append(_p)

from contextlib import ExitStack

import numpy as np

import concourse.bass as bass
import concourse.mybir as mybir
from concourse.bass import _add_dep_helper
import concourse.tile as tile
from concourse import bacc
from concourse.bass_utils import run_bass_kernel_spmd

B, L, C, H, D = 4, 1024, 512, 2048, 128
NHL = 8          # heads per core
NPAIR = 4        # head pairs per core
T = L            # tokens per core (one sequence)
EPS = 1e-5
f32 = mybir.dt.float32
f32r = mybir.dt.float32r
bf16 = mybir.dt.bfloat16

_CACHE = {}


def _build(debug=False, collective=True, nbody=1, nrs=1):
    nc = bacc.Bacc("TRN2", target_bir_lowering=False, debug=False, num_devices=8)

    x_t = nc.dram_tensor("x", [T, H], f32r, kind="ExternalInput")
    wq_t = nc.dram_tensor("wq", [H, NHL * D], f32r, kind="ExternalInput")
    wk_t = nc.dram_tensor("wk", [H, NHL * D], f32r, kind="ExternalInput")
    wv_t = nc.dram_tensor("wv", [H, NHL * D], f32r, kind="ExternalInput")
    rowsa_t = nc.dram_tensor("rowsa", [4, 3 * NHL * D], f32r, kind="ExternalInput")
    rowsb_t = nc.dram_tensor("rowsb", [4, T], f32r, kind="ExternalInput")
    kc_t = nc.dram_tensor("kc", [C, NHL, D], f32r, kind="ExternalInput")
    vc_t = nc.dram_tensor("vc", [C, NHL, D], f32r, kind="ExternalInput")
    pw_t = nc.dram_tensor("pw", [NHL * D, H], bf16, kind="ExternalInput")
    pb_t = nc.dram_tensor("pb", [1, H], f32, kind="ExternalInput")
    id_t = nc.dram_tensor("ident", [128, 128], f32r, kind="ExternalInput")
    mask_t = nc.dram_tensor("mask", [128, 896], f32r, kind="ExternalInput")
    onc_t = nc.dram_tensor("onescol", [128, 1], f32r, kind="ExternalInput")
    out_t = nc.dram_tensor("out", [T // 2, H], f32, kind="ExternalOutput")
    if debug:
        dbg_xt = nc.dram_tensor("dbg_xt", [128, 16, T], f32, kind="ExternalOutput")
        dbg_q = nc.dram_tensor("dbg_q", [128, 2, T], f32, kind="ExternalOutput")
        dbg_k = nc.dram_tensor("dbg_k", [128, 2, T], f32, kind="ExternalOutput")
        dbg_v = nc.dram_tensor("dbg_v", [128, 8, 256], f32, kind="ExternalOutput")
        dbg_ot = nc.dram_tensor("dbg_ot", [128, NHL, T], bf16,
                                kind="ExternalOutput")
        dbg_pp = nc.dram_tensor("dbg_pp", [T, H], f32, kind="ExternalOutput")

    Exp = mybir.ActivationFunctionType.Exp
    Sqrt = mybir.ActivationFunctionType.Sqrt
    Ident = mybir.ActivationFunctionType.Identity
    mult = mybir.AluOpType.mult

    with tile.TileContext(nc) as tc, ExitStack() as _stk:
        def _pool(name, bufs, **kw):
            return _stk.enter_context(tc.tile_pool(name=name, bufs=bufs, **kw))
        cst = _pool("cst", 1)
        xload = _pool("xload", 4)
        st = _pool("st", 8)
        big = _pool("big", 1)
        wp = _pool("wp", 4)
        qkvp = _pool("qkv", 3)
        kvc = _pool("kvc", 2)
        kcnp = _pool("kcn", 2)
        ptp = _pool("ptp", 2)
        rb = _pool("rb", 1)
        otp = _pool("otp", 1)
        po = _pool("po", 2)
        ps = _pool("ps", 8, space="PSUM")
        dram = _pool("dram", 1, space="DRAM")

        # ---- constants ----
        ident = cst.tile([128, 128], f32r)
        nc.sync.dma_start(out=ident, in_=id_t.ap())
        mask = cst.tile([128, 896], f32r)
        nc.sync.dma_start(out=mask, in_=mask_t.ap())
        rowsa = cst.tile([4, 3 * NHL * D], f32r)   # 0, const, 0, 0
        nc.sync.dma_start(out=rowsa, in_=rowsa_t.ap())
        rowsb = cst.tile([4, T], f32r)             # 0, ones, 0, 0
        nc.sync.dma_start(out=rowsb, in_=rowsb_t.ap())
        onescol = cst.tile([128, 1], f32r)
        nc.sync.dma_start(out=onescol, in_=onc_t.ap())
        epst = cst.tile([128, 1], f32)
        nc.vector.memset(epst, EPS)

        partial = dram.tile([T, H], bf16)
        rsout = dram.tile([T // 2, H], bf16)

        for _rep in range(nbody):
            xT = big.tile([128, 16, T], f32r, tag="big")

            def ln_chunk(tch):
                """LN one 128-token chunk and transpose into xT."""
                xh = [xload.tile([128, 1024], f32r, tag="x", name=f"xh{i_}")
                      for i_ in range(2)]
                stats = st.tile([128, 4, 6], f32, tag="stats")
                for half in range(2):
                    d = nc.sync.dma_start(
                        out=xh[half],
                        in_=x_t.ap()[tch * 128:(tch + 1) * 128,
                                     half * 1024:(half + 1) * 1024])
                    x_dmas.append(d)
                    for sg in range(2):
                        nc.vector.bn_stats(
                            out=stats[:, half * 2 + sg, :],
                            in_=xh[half][:, sg * 512:(sg + 1) * 512].bitcast(f32))
                mv = st.tile([128, 2], f32, tag="mv")
                nc.vector.bn_aggr(out=mv, in_=stats)
                stdt = st.tile([128, 1], f32, tag="std")
                nc.scalar.activation(stdt, mv[:, 1:2], Sqrt, bias=epst[:, 0:1])
                rstd = st.tile([128, 1], f32, tag="rstd")
                nc.vector.reciprocal(rstd, stdt)
                m2p = st.tile([128, 1], f32, tag="m2p")
                nc.vector.tensor_scalar(
                    out=m2p, in0=mv[:, 0:1],
                    scalar1=rstd[:, 0:1], scalar2=-1.0, op0=mult, op1=mult)
                for half in range(2):
                    nc.scalar.activation(
                        xh[half][:], xh[half][:].bitcast(f32), Ident,
                        bias=m2p[:, 0:1], scale=rstd[:, 0:1])
                for hch in range(16):
                    tp = ps.tile([128, 512], f32, tag="ps", name="tp")
                    nc.tensor.transpose(
                        tp[:, 0:128].bitcast(f32r),
                        xh[hch // 8][:, (hch % 8) * 128:(hch % 8 + 1) * 128],
                        ident)
                    nc.vector.tensor_copy(
                        out=xT[:, hch, tch * 128:(tch + 1) * 128],
                        in_=tp[:, 0:128].bitcast(f32r))

            def load_w(wdram, hp, after=()):
                wh = []
                for half in range(2):
                    w = wp.tile([128, 8, 256], f32r, tag="w", name=f"w{half}")
                    wh.append(w)
                    d = nc.scalar.dma_start(
                        out=w,
                        in_=wdram.ap()[half * 1024:(half + 1) * 1024,
                                       hp * 256:(hp + 1) * 256].rearrange(
                            "(c p) n -> p c n", p=128))
                    for a in after:
                        _add_dep_helper(d.ins, a.ins,
                                        reason="delay W behind x loads")
                return wh

            def emit_qk(dst, wh, colbase, jh, tt):
                pq = ps.tile([128, 512], f32, tag="ps", name="pq")
                for half in range(2):
                    for hc in range(8):
                        nc.tensor.matmul(
                            pq[:],
                            wh[half][:, hc, jh * 128:(jh + 1) * 128],
                            xT[:, half * 8 + hc, tt * 512:(tt + 1) * 512],
                            start=(half == 0 and hc == 0), stop=False)
                nc.tensor.matmul(
                    pq[:],
                    rowsa[0:4, colbase + jh * 128:colbase + (jh + 1) * 128],
                    rowsb[0:4, tt * 512:(tt + 1) * 512],
                    start=False, stop=True)
                nc.vector.tensor_copy(
                    out=dst[:, jh, tt * 512:(tt + 1) * 512],
                    in_=pq[:].bitcast(f32r))

            def emit_v(vtok, whv, vcol, tch):
                pv = ps.tile([128, 512], f32, tag="ps", name="pv")
                for half in range(2):
                    for hc in range(8):
                        nc.tensor.matmul(
                            pv[:, 0:256],
                            xT[:, half * 8 + hc, tch * 128:(tch + 1) * 128],
                            whv[half][:, hc, :],
                            start=(half == 0 and hc == 0), stop=False)
                nc.tensor.matmul(
                    pv[:, 0:256],
                    rowsb[0:4, tch * 128:(tch + 1) * 128],
                    rowsa[0:4, vcol:vcol + 256],
                    start=False, stop=True)
                nc.vector.tensor_copy(out=vtok[:, tch, :],
                                      in_=pv[:, 0:256].bitcast(f32r))

            def load_cache(hp):
                kcT = kvc.tile([128, 2, C], f32r, tag="kv", name="kcT")
                for pc in range(4):
                    kcn = kcnp.tile([128, 2, 128], f32r, tag="kcn", name="kcn")
                    nc.scalar.dma_start(
                        out=kcn,
                        in_=kc_t.ap()[pc * 128:(pc + 1) * 128,
                                      hp * 2:hp * 2 + 2, :])
                    tp = ps.tile([128, 512], f32, tag="ps", name="tpc")
                    for hl in range(2):
                        nc.tensor.transpose(
                            tp[:, hl * 128:(hl + 1) * 128].bitcast(f32r),
                            kcn[:, hl, :], ident)
                    for hl in range(2):
                        nc.vector.tensor_copy(
                            out=kcT[:, hl, pc * 128:(pc + 1) * 128],
                            in_=tp[:, hl * 128:(hl + 1) * 128].bitcast(f32r))
                vcp = kvc.tile([128, 4, 2, 128], f32r, tag="kv", name="vcp")
                nc.scalar.dma_start(
                    out=vcp,
                    in_=vc_t.ap()[:, hp * 2:hp * 2 + 2, :].rearrange(
                        "(c p) h d -> p c h d", p=128))
                return kcT, vcp

            def emit_attn(outT, qT, kT, vtok, kcT, vcp, hp, hl, qt):
                h = hp * 2 + hl
                po_ps = ps.tile([128, 512], f32, tag="ps", name="po_ps")
                sm_ps = ps.tile([128, 512], f32, tag="ps", name="sm_ps")
                chunks = [("c", pc, None) for pc in range(4)]
                for kf in range(8):
                    dlt = qt * 512 - kf * 128
                    if dlt <= -512:
                        continue
                    chunks.append(("f", kf, dlt + 384 if dlt < 127 else None))
                first = True
                for kind, idx, ms in chunks:
                    sc_ps = ps.tile([128, 512], f32, tag="ps", name="sc_ps")
                    lhs = (kcT[:, hl, idx * 128:(idx + 1) * 128] if kind == "c"
                           else kT[:, hl, idx * 128:(idx + 1) * 128])
                    nc.tensor.matmul(
                        sc_ps[:], lhs, qT[:, hl, qt * 512:(qt + 1) * 512],
                        start=True, stop=True)
                    pt = ptp.tile([128, 512], f32r, tag="pt", name="pt")
                    nc.scalar.activation(pt[:], sc_ps[:], Exp)
                    if ms is not None:
                        nc.vector.tensor_mul(pt[:], pt[:], mask[:, ms:ms + 512])
                    vchunk = (vcp[:, idx, hl, :] if kind == "c"
                              else vtok[:, idx, hl * 128:(hl + 1) * 128])
                    nc.tensor.matmul(po_ps[:], vchunk, pt[:],
                                     start=first, stop=False)
                    nc.tensor.matmul(sm_ps[0:1, :], onescol, pt[:],
                                     start=first, stop=False)
                    first = False
                rc = st.tile([1, 512], f32, tag="recip", bufs=2)
                nc.vector.reciprocal(rc, sm_ps[0:1, :])
                rcb = rb.tile([128, 512], f32, tag="rcb", name="rcb")
                nc.gpsimd.partition_broadcast(rcb[:], rc[:])
                nc.vector.tensor_mul(
                    out=outT[:, h, qt * 512:(qt + 1) * 512],
                    in0=po_ps[:], in1=rcb[:])

            # ---- phase A interleaved with pair-0 QKV ----
            x_dmas = []
            for tch in range(2):
                ln_chunk(tch)
            first_x = list(x_dmas)
            wh_q = load_w(wq_t, 0, after=first_x)
            wh_v = load_w(wv_t, 0, after=first_x)
            for tch in range(2, 4):
                ln_chunk(tch)
            outT = otp.tile([128, NHL, T], bf16, tag="outT")
            qT0 = qkvp.tile([128, 2, T], f32r, tag="qkv", name="qT0")
            vtok0 = qkvp.tile([128, 8, 256], f32r, tag="qkv", name="vtok0")
            for jh in range(2):
                emit_qk(qT0, wh_q, 0, jh, 0)
            for tch in range(4):
                emit_v(vtok0, wh_v, 2048, tch)
            for tch in range(4, 8):
                ln_chunk(tch)
            for jh in range(2):
                emit_qk(qT0, wh_q, 0, jh, 1)
            for tch in range(4, 8):
                emit_v(vtok0, wh_v, 2048, tch)
            wh_k = load_w(wk_t, 0)
            kT0 = qkvp.tile([128, 2, T], f32r, tag="qkv", name="kT0")
            for jh in range(2):
                for tt in range(2):
                    emit_qk(kT0, wh_k, 1024, jh, tt)

            if debug:
                nc.sync.dma_start(out=dbg_xt.ap(), in_=xT[:].bitcast(f32))
                nc.sync.dma_start(out=dbg_q.ap(), in_=qT0[:].bitcast(f32))
                nc.sync.dma_start(out=dbg_k.ap(), in_=kT0[:].bitcast(f32))
                nc.sync.dma_start(out=dbg_v.ap(), in_=vtok0[:].bitcast(f32))

            # ---- phase B ----
            for hp in range(NPAIR):
                if hp == 0:
                    qT, kT, vtok = qT0, kT0, vtok0
                else:
                    wh = load_w(wq_t, hp)
                    qT = qkvp.tile([128, 2, T], f32r, tag="qkv", name="qT")
                    for jh in range(2):
                        for tt in range(2):
                            emit_qk(qT, wh, hp * 256, jh, tt)
                    wh = load_w(wk_t, hp)
                    kT = qkvp.tile([128, 2, T], f32r, tag="qkv", name="kT")
                    for jh in range(2):
                        for tt in range(2):
                            emit_qk(kT, wh, 1024 + hp * 256, jh, tt)
                    wh = load_w(wv_t, hp)
                    vtok = qkvp.tile([128, 8, 256], f32r, tag="qkv", name="vtok")
                    for tch in range(8):
                        emit_v(vtok, wh, 2048 + hp * 256, tch)

                if hp == NPAIR - 1:
                    pw_sb = big.tile([128, NHL, H], bf16, tag="big")
                    for hq_ in range(4):
                        nc.scalar.dma_start(
                            out=pw_sb[:, hq_ * 2:(hq_ + 1) * 2, :],
                            in_=pw_t.ap()[hq_ * 256:(hq_ + 1) * 256, :
                                          ].rearrange("(h p) n -> p h n", p=128))
                    pbb = wp.tile([128, H], f32, tag="w", name="pbb")
                    nc.gpsimd.dma_start(
                        out=pbb,
                        in_=bass.AP(tensor=pb_t.ap().tensor, offset=0,
                                    ap=[[0, 128], [1, H]]))

                kcT, vcp = load_cache(hp)
                for hl in range(2):
                    for qt in range(2):
                        emit_attn(outT, qT, kT, vtok, kcT, vcp, hp, hl, qt)

            if debug:
                nc.sync.dma_start(out=dbg_ot.ap(), in_=outT[:])

            # ---- phase C: c_proj partials (bf16) ----
            for tch in range(8):
                for ht in range(4):
                    pp = ps.tile([128, 512], f32, tag="ps", name="pp")
                    for h in range(NHL):
                        nc.tensor.matmul(
                            pp[:],
                            outT[:, h, tch * 128:(tch + 1) * 128],
                            pw_sb[:, h, ht * 512:(ht + 1) * 512],
                            start=(h == 0), stop=(h == NHL - 1))
                    ev = po.tile([128, 512], bf16, tag="po", name="ev")
                    nc.vector.tensor_add(ev[:], pp[:],
                                         pbb[:, ht * 512:(ht + 1) * 512])
                    nc.sync.dma_start(
                        out=partial[tch * 128:(tch + 1) * 128,
                                    ht * 512:(ht + 1) * 512],
                        in_=ev[:])

            if debug:
                nc.gpsimd.dma_start(out=dbg_pp.ap(), in_=partial[:, :])

        # ---- phase D: ReduceScatter over pairs, write out ----
        if collective:
            for _ in range(nrs):
                nc.gpsimd.collective_compute(
                    "ReduceScatter",
                    mybir.AluOpType.add,
                    replica_groups=[[0, 1], [2, 3], [4, 5], [6, 7]],
                    ins=[partial.opt()],
                    outs=[rsout.opt()],
                )
            nc.gpsimd.dma_start(out=out_t.ap(), in_=rsout[:, :])
        else:
            nc.gpsimd.dma_start(out=out_t.ap(), in_=partial[0:T // 2, :])

    nc.compile()
    return nc


def _host_prep(inputs):
    import ml_dtypes
    hidden = np.ascontiguousarray(np.asarray(inputs["hidden_states"],
                                             dtype=np.float32))
    k_cache = np.asarray(inputs["k_cache"], dtype=np.float32)
    v_cache = np.asarray(inputs["v_cache"], dtype=np.float32)
    ln_w = np.asarray(inputs["ln_w"], dtype=np.float32)
    ln_b = np.asarray(inputs["ln_b"], dtype=np.float32)
    attn_w = np.asarray(inputs["attn_w"], dtype=np.float32)
    attn_b = np.asarray(inputs["attn_b"], dtype=np.float32)
    proj_w = np.asarray(inputs["proj_w"], dtype=np.float32)
    proj_b = np.asarray(inputs["proj_b"], dtype=np.float32)

    scale = np.float32(1.0 / np.sqrt(D))
    wln = attn_w * ln_w[:, None]                  # [H, 6144]
    cfull = ln_b @ attn_w + attn_b                # [6144]
    ident = np.eye(128, dtype=np.float32)
    cc = np.arange(896)[None, :] - 384
    mask = (np.arange(128)[:, None] <= cc).astype(np.float32)
    onescol = np.ones((128, 1), dtype=np.float32)
    rowsb = np.zeros((4, T), dtype=np.float32)
    rowsb[1] = 1.0

    in_maps = []
    for c in range(8):
        b, g = c // 2, c % 2
        hsl = slice(g * NHL, (g + 1) * NHL)
        qsl = slice(g * 1024, (g + 1) * 1024)
        ksl = slice(2048 + g * 1024, 2048 + (g + 1) * 1024)
        vsl = slice(4096 + g * 1024, 4096 + (g + 1) * 1024)
        wq = np.ascontiguousarray(wln[:, qsl]) * scale
        wk = np.ascontiguousarray(wln[:, ksl])
        wv = np.ascontiguousarray(wln[:, vsl])
        const = np.concatenate([cfull[qsl] * scale, cfull[ksl], cfull[vsl]])
        rowsa = np.zeros((4, 3072), dtype=np.float32)
        rowsa[1] = const
        in_maps.append({
            "x": np.ascontiguousarray(hidden[b * L:(b + 1) * L]),
            "wq": wq, "wk": wk, "wv": wv,
            "rowsa": rowsa, "rowsb": rowsb,
            "kc": np.ascontiguousarray(k_cache[b, :C, hsl, :]),
            "vc": np.ascontiguousarray(v_cache[b, :C, hsl, :]),
            "pw": np.ascontiguousarray(proj_w[qsl, :]).astype(ml_dtypes.bfloat16),
            "pb": (proj_b if g == 0 else np.zeros_like(proj_b)
                   ).reshape(1, H).astype(np.float32),
            "ident": ident, "mask": mask, "onescol": onescol,
        })
    return in_maps


def kernel(**inputs) -> np.ndarray:
    if "nc" not in _CACHE:
        _CACHE["nc"] = _build()
    nc = _CACHE["nc"]
    in_maps = _host_prep(inputs)
    res = run_bass_kernel_spmd(nc, in_maps, list(range(8)))
    out = np.concatenate([res.results[c]["out"] for c in range(8)], axis=0)
    return out.astype(np.float32)
